# revision 24
# baseline (speedup 1.0000x reference)
"""GCNConv(flow=target_to_source) + BatchNorm + ReLU + residual, on 8 trn2 NeuronCores.

Math: with self-loops,
    deg[i]   = 1 + #{e : row[e] == i}
    dinv     = deg ** -0.5
    v        = dinv[:, None] * x                      (bf16 table, HOST-precomputed input)
    S[i]     = v[i] + sum_{e: row[e]=i} v[col[e]]     (dma_gather + onehot-matmul scatter)
    out      = dinv[:, None] * (S @ W)                (dinv applied pre-W: commutes)
    y        = relu((out - mean) * rsqrt(var + eps) * gamma + beta) + x
(b cancels inside BatchNorm and is dropped.)

Sharding: nodes (rows) split across 8 cores; edges partitioned by destination
row so the scatter-add is core-local PSUM accumulation.  Rows within a core are
PERMUTED into 50 blocks of 125 by a load-balancing greedy pack so every block
needs exactly t_lo=11 lo-tiles and t_hi=6 hi-tiles (the permutation is undone
on the host).  BN statistics go through a [128,2] AllReduce.

Minimal-instruction-count structure (HW has a large per-instruction cost on the
DVE/ACT engines): per 5-block chunk there is ONE mega-fused onehot build, ONE
fused (S + self-loop)*dinv pair, ONE batched PSUM->SBUF copy; the only
per-block ops are the PE matmuls (HW-decoded, cheap).

dma_gather takes int16 indices, so the v table is addressed as two halves
(lo: rows < 32768, hi: rows >= 32768).  Index buffers use the HW layout:
idx i at (partition i%16, column i//16), replicated across the eight
16-partition groups.
"""

import os
import sys

sys.path.insert(0, "/opt/trn_rl_repo")
os.environ.setdefault("MYCRO_LOCAL_CACHE", "1")

from contextlib import ExitStack

import ml_dtypes
import numpy as np

CORES = 8
BN_EPS = 1e-5
SPLIT = 32768
N_NODES = 50000
DIM = 128
NPC = N_NODES // CORES        # 6250
BLK = 125
NBLK = NPC // BLK             # 50
T_LO = 11                     # lo tiles per block (cap 1408 edges)
T_HI = 6                      # hi tiles per block (cap 768 edges)
T = T_LO + T_HI               # 17
SUP = 5                       # blocks per gather chunk
_CACHE: dict = {}


def _strided(ap_src, offset_elems, dims):
    import concourse.bass as bass

    return bass.AP(ap_src.tensor, offset_elems, [list(d) for d in dims])


def _build_nc():
    from concourse import bacc, bass, mybir, tile
    from concourse.masks import make_identity

    f32 = mybir.dt.float32
    bf16 = mybir.dt.bfloat16
    i16 = mybir.dt.int16
    D = DIM

    nc = bacc.Bacc(
        "TRN2",
        target_bir_lowering=False,
        debug=False,
        enable_asserts=False,
        num_devices=CORES,
        num_swdge_queues=4,
    )

    v_t = nc.dram_tensor("v_tab", [N_NODES, D], bf16, kind="ExternalInput").ap()
    lo_t = nc.dram_tensor("lo_idx", [128, NBLK * T_LO * 8], i16, kind="ExternalInput").ap()
    hi_t = nc.dram_tensor("hi_idx", [128, NBLK * T_HI * 8], i16, kind="ExternalInput").ap()
    rel_t = nc.dram_tensor("rel_arr", [128, NBLK * T], bf16, kind="ExternalInput").ap()
    iota_t = nc.dram_tensor("iota_rt", [128, BLK * T], bf16, kind="ExternalInput").ap()
    vlt_t = nc.dram_tensor("v_loc_t", [128, NPC], bf16, kind="ExternalInput").ap()
    dvr_t = nc.dram_tensor("dinvrow", [128, NPC], bf16, kind="ExternalInput").ap()
    xloc_t = nc.dram_tensor("x_loc", [BLK, NBLK * D], bf16, kind="ExternalInput").ap()
    w_t = nc.dram_tensor("w_mat", [D, D], bf16, kind="ExternalInput").ap()
    gb_t = nc.dram_tensor("gb", [128, 2], f32, kind="ExternalInput").ap()
    y_t = nc.dram_tensor("y_out", [BLK, NBLK * D], bf16, kind="ExternalOutput").ap()

    NCHUNK = NBLK // SUP
    LO_C = SUP * T_LO * 8   # idx cols per chunk (lo)
    HI_C = SUP * T_HI * 8

    with tile.TileContext(nc) as tc, ExitStack() as ctx:
        const = ctx.enter_context(tc.tile_pool(name="const", bufs=1))
        idxp = ctx.enter_context(tc.tile_pool(name="idxp", bufs=1))
        gath = ctx.enter_context(tc.tile_pool(name="gath", bufs=3))
        ohp = ctx.enter_context(tc.tile_pool(name="ohp", bufs=2))
        evp = ctx.enter_context(tc.tile_pool(name="evp", bufs=2))
        big = ctx.enter_context(tc.tile_pool(name="big", bufs=1))
        ps_main = ctx.enter_context(tc.tile_pool(name="ps_main", bufs=2, space="PSUM"))
        ps_ow = ctx.enter_context(tc.tile_pool(name="ps_ow", bufs=1, space="PSUM"))
        ps_stat = ctx.enter_context(tc.tile_pool(name="ps_stat", bufs=1, space="PSUM"))
        dram = ctx.enter_context(tc.tile_pool(name="dram", bufs=1, space="DRAM"))

        # ---- index tiles: chunk-0 slice first (fast first gather), rest bulk
        lo0_sb = idxp.tile([128, LO_C], i16, tag="lo0")
        nc.sync.dma_start(lo0_sb[:], lo_t[:, 0:LO_C])
        hi0_sb = idxp.tile([128, HI_C], i16, tag="hi0")
        nc.sync.dma_start(hi0_sb[:], hi_t[:, 0:HI_C])
        lor_sb = idxp.tile([128, (NCHUNK - 1) * LO_C], i16, tag="lor")
        nc.sync.dma_start(lor_sb[:], lo_t[:, LO_C:NCHUNK * LO_C])
        hir_sb = idxp.tile([128, (NCHUNK - 1) * HI_C], i16, tag="hir")
        nc.sync.dma_start(hir_sb[:], hi_t[:, HI_C:NCHUNK * HI_C])

        def issue_gather(c):
            if c == 0:
                lo_ap, hi_ap = lo0_sb[:], hi0_sb[:]
            else:
                lo_ap = lor_sb[:, (c - 1) * LO_C:c * LO_C]
                hi_ap = hir_sb[:, (c - 1) * HI_C:c * HI_C]
            g = gath.tile([128, SUP * T, D], bf16)
            nc.gpsimd.dma_gather(
                g[:, 0:SUP * T_LO, :],
                v_t[0:SPLIT, :],
                lo_ap,
                SUP * T_LO * 128,
                SUP * T_LO * 128,
                D,
                single_packet=False,
                queue_num=(2 * c) % 4,
            )
            nc.gpsimd.dma_gather(
                g[:, SUP * T_LO:SUP * T, :],
                v_t[SPLIT:N_NODES, :],
                hi_ap,
                SUP * T_HI * 128,
                SUP * T_HI * 128,
                D,
                single_packet=False,
                queue_num=(2 * c + 1) % 4,
            )
            return g

        g_tiles = {0: issue_gather(0)}

        # ---- constants (after chunk-0 gather is in flight) ------------------
        w_sb = const.tile([D, D], bf16)
        nc.sync.dma_start(w_sb[:], w_t[:])
        iota_sb = const.tile([128, BLK * T], bf16)
        nc.sync.dma_start(iota_sb[:], iota_t[:])
        rel_sb = const.tile([128, NBLK * T], bf16)
        nc.sync.dma_start(rel_sb[:], rel_t[:])
        vlt_sb = const.tile([128, NPC], bf16)
        nc.sync.dma_start(vlt_sb[:], vlt_t[:])
        dvr_sb = const.tile([128, NPC], bf16)
        nc.sync.dma_start(dvr_sb[:], dvr_t[:])
        gb_sb = const.tile([128, 2], f32)
        nc.sync.dma_start(gb_sb[:], gb_t[:])
        ones_sb = const.tile([BLK, 1], bf16)
        nc.vector.memset(ones_sb[:], 1.0 / float(N_NODES))
        onesrow_sb = const.tile([1, 128], f32)
        nc.vector.memset(onesrow_sb[:], 1.0)
        ident_sb = const.tile([128, 128], f32)
        make_identity(nc, ident_sb[:])
        identinv_sb = const.tile([128, 128], f32)
        nc.vector.tensor_scalar(
            out=identinv_sb[:], in0=ident_sb[:], scalar1=1.0 / float(N_NODES),
            scalar2=None, op0=mybir.AluOpType.mult,
        )
        xl = big.tile([128, NBLK * D], bf16)

        out_all = big.tile([128, NBLK * D], bf16)
        s1t = ps_stat.tile([128, 1], f32, tag="s1")
        s2mt = ps_stat.tile([128, 128], f32, tag="s2m")
        s1 = s1t[:]
        s2m = s2mt[:]

        # ---- main loop: SUP blocks per gather chunk -------------------------
        g_tiles[1] = issue_gather(1)
        for c in range(NCHUNK):
            if c + 2 < NCHUNK:
                g_tiles[c + 2] = issue_gather(c + 2)
            if c == 1:
                nc.sync.dma_start(xl[:BLK, :], xloc_t[:])
            g = g_tiles.pop(c)

            # ONE mega-fused onehot for the whole chunk: [128, SUP, BLK, T]
            oh = ohp.tile([128, SUP * BLK * T], bf16)
            iota_rep = _strided(
                iota_sb[:], 0,
                [list(iota_sb[:].ap[0]), [0, SUP], [1, BLK * T]],
            )
            rel_rep = _strided(
                rel_sb[:], c * SUP * T,
                [list(rel_sb[:].ap[0]), [T, SUP], [0, BLK], [1, T]],
            )
            nc.vector.tensor_tensor(
                out=oh[:], in0=iota_rep, in1=rel_rep, op=mybir.AluOpType.is_equal
            )

            st = ps_main.tile([128, SUP, 128], f32, tag="st")
            for j in range(SUP):
                for t in range(T):
                    if t < T_LO:
                        src = g[:, j * T_LO + t, :]
                    else:
                        src = g[:, SUP * T_LO + j * T_HI + (t - T_LO), :]
                    rhs = _strided(
                        oh[:], j * BLK * T + t, [list(oh[:].ap[0]), [T, BLK]]
                    )
                    nc.tensor.matmul(
                        out=st[:, j, 0:BLK], lhsT=src, rhs=rhs,
                        start=(t == 0), stop=(t == T - 1),
                    )

            # fused (S + self-loop) * dinv for the whole chunk (2 DVE ops)
            stb0 = evp.tile([128, SUP * BLK], bf16, tag="stb0")
            st_view = _strided(st[:], 0, [list(st[:].ap[0]), [128, SUP], [1, BLK]])
            nc.vector.tensor_tensor(
                out=stb0[:], in0=st_view,
                in1=vlt_sb[:, c * SUP * BLK:(c + 1) * SUP * BLK],
                op=mybir.AluOpType.add,
            )
            stb = evp.tile([128, SUP * BLK], bf16, tag="stb")
            nc.vector.tensor_tensor(
                out=stb[:], in0=stb0[:],
                in1=dvr_sb[:, c * SUP * BLK:(c + 1) * SUP * BLK],
                op=mybir.AluOpType.mult,
            )

            ow = ps_ow.tile([BLK, SUP, D], f32, tag="ow")
            for j in range(SUP):
                nc.tensor.matmul(
                    out=ow[:, j, :], lhsT=stb[:, j * BLK:(j + 1) * BLK],
                    rhs=w_sb[:], start=True, stop=True,
                )
            # ONE batched PSUM->SBUF copy per chunk (ACT engine)
            cslice = out_all[:BLK, c * SUP * D:(c + 1) * SUP * D]
            nc.scalar.activation(
                out=cslice, in_=ow[:, :, :],
                func=mybir.ActivationFunctionType.Copy,
            )
            # stats per block (PE): s1 += oslice^T @ (1/N); s2m += oslice^T @ oslice
            for j in range(SUP):
                blk = c * SUP + j
                oslice = out_all[:BLK, blk * D:(blk + 1) * D]
                nc.tensor.matmul(
                    out=s1, lhsT=oslice, rhs=ones_sb[:],
                    start=(blk == 0), stop=(blk == NBLK - 1),
                )
                nc.tensor.matmul(
                    out=s2m, lhsT=oslice, rhs=oslice,
                    start=(blk == 0), stop=(blk == NBLK - 1),
                )

        # ---- BN stats AllReduce + affine params -----------------------------
        stat_sb = const.tile([128, 2], f32, name="stat_sb")
        nc.vector.tensor_copy(out=stat_sb[:, 0:1], in_=s1)
        s2mask = evp.tile([128, 128], f32, tag="s2mask")
        nc.vector.tensor_tensor(
            out=s2mask[:], in0=s2m, in1=identinv_sb[:], op=mybir.AluOpType.mult
        )
        nc.vector.tensor_reduce(
            out=stat_sb[:, 1:2], in_=s2mask[:],
            axis=mybir.AxisListType.X, op=mybir.AluOpType.add,
        )
        cc_in = dram.tile([128, 2], f32)
        cc_out = dram.tile([128, 2], f32, addr_space="Shared")
        nc.sync.dma_start(cc_in[:], stat_sb[:])
        nc.gpsimd.collective_compute(
            "AllReduce",
            mybir.AluOpType.add,
            replica_groups=[list(range(CORES))],
            ins=[cc_in.opt()],
            outs=[cc_out.opt()],
        )
        stats_g = const.tile([128, 2], f32, name="stats_g")
        nc.sync.dma_start(stats_g[:], cc_out[:])

        mean = stats_g[:, 0:1]
        vareps = const.tile([128, 1], f32, name="vareps")
        m2 = const.tile([128, 1], f32, name="m2")
        nc.vector.tensor_tensor(out=m2[:], in0=mean, in1=mean, op=mybir.AluOpType.mult)
        nc.vector.tensor_scalar(
            out=vareps[:], in0=stats_g[:, 1:2], scalar1=BN_EPS, scalar2=None,
            op0=mybir.AluOpType.add,
        )
        nc.vector.tensor_tensor(
            out=vareps[:], in0=vareps[:], in1=m2[:], op=mybir.AluOpType.subtract
        )
        rec1 = const.tile([128, 1], f32, name="rec1")
        nc.vector.reciprocal(out=rec1[:], in_=vareps[:])
        rsq = const.tile([128, 1], f32, name="rsq")
        nc.scalar.sqrt(out=rsq[:], in_=rec1[:])
        ab_sb = const.tile([128, 2], f32, name="ab_sb")
        nc.vector.tensor_tensor(
            out=ab_sb[:, 0:1], in0=rsq[:], in1=gb_sb[:, 0:1], op=mybir.AluOpType.mult
        )
        tmb = const.tile([128, 1], f32, name="tmb")
        nc.vector.tensor_tensor(
            out=tmb[:], in0=mean, in1=ab_sb[:, 0:1], op=mybir.AluOpType.mult
        )
        nc.vector.tensor_tensor(
            out=ab_sb[:, 1:2], in0=gb_sb[:, 1:2], in1=tmb[:], op=mybir.AluOpType.subtract
        )

        def bcast_col(col_ap, nm):
            tp = ps_ow.tile([128, 128], f32, tag="ow")
            nc.tensor.transpose(out=tp[:1, :], in_=col_ap, identity=ident_sb[:])
            rowt = const.tile([1, 128], f32, name=f"rowt_{nm}")
            nc.vector.tensor_copy(out=rowt[:], in_=tp[:1, :])
            bc_ps = ps_ow.tile([128, 128], f32, tag="ow")
            nc.tensor.matmul(out=bc_ps[:], lhsT=onesrow_sb[:], rhs=rowt[:], start=True, stop=True)
            bc = const.tile([128, 128], bf16, name=f"bc_{nm}")
            nc.vector.tensor_copy(out=bc[:], in_=bc_ps[:])
            return bc

        a_bc = bcast_col(ab_sb[:, 0:1], "a")
        b_bc = bcast_col(ab_sb[:, 1:2], "b")

        # ---- final apply: y = relu(out*A + B) + x  (all bf16, in place) -----
        a_rep = _strided(a_bc[:], 0, [[a_bc[:].ap[0][0], BLK], [0, NBLK], [1, D]])
        b_rep = _strided(b_bc[:], 0, [[b_bc[:].ap[0][0], BLK], [0, NBLK], [1, D]])
        nc.vector.tensor_tensor(
            out=out_all[:BLK, :], in0=out_all[:BLK, :], in1=a_rep, op=mybir.AluOpType.mult
        )
        nc.vector.tensor_tensor(
            out=out_all[:BLK, :], in0=out_all[:BLK, :], in1=b_rep, op=mybir.AluOpType.add
        )
        nc.vector.tensor_scalar(
            out=out_all[:BLK, :], in0=out_all[:BLK, :], scalar1=0.0, scalar2=None,
            op0=mybir.AluOpType.max,
        )
        nc.vector.tensor_tensor(
            out=out_all[:BLK, :], in0=out_all[:BLK, :], in1=xl[:BLK, :],
            op=mybir.AluOpType.add,
        )
        nc.sync.dma_start(y_t[:], out_all[:BLK, :])

    nc.compile()
    return nc


def _pack_idx(vals_by_seg, n_tiles, nblk):
    """Pack per-block index segments into the dma_gather int16 layout:
    idx i -> (partition i%16, col i//16), replicated across the 8 groups
    of 16 partitions.  Returns [128, nblk * n_tiles * 8] int16."""
    ncols = n_tiles * 8
    out = np.zeros((128, nblk * ncols), np.int16)
    for b, vals in enumerate(vals_by_seg):
        padded = np.zeros(n_tiles * 128, np.int16)
        padded[: len(vals)] = vals
        grid = padded.reshape(ncols, 16).T  # [16, ncols]
        out[:, b * ncols:(b + 1) * ncols] = np.tile(grid, (8, 1))
    return out


def _balance_blocks(lo_deg, hi_deg):
    """Greedily pack NPC rows into NBLK blocks of exactly BLK rows so that
    every block's lo/hi edge counts stay under T_LO*128 / T_HI*128.
    Returns (block_of_row, pos_of_row)."""
    order = np.argsort(-(lo_deg + hi_deg), kind="stable")
    lo_load = np.zeros(NBLK)
    hi_load = np.zeros(NBLK)
    cnt = np.zeros(NBLK, dtype=np.int64)
    block_of = np.empty(NPC, np.int64)
    pos_of = np.empty(NPC, np.int64)
    lo_cap = T_LO * 128.0
    hi_cap = T_HI * 128.0
    for r in order:
        score = np.maximum(
            (lo_load + lo_deg[r]) / lo_cap, (hi_load + hi_deg[r]) / hi_cap
        )
        score[cnt >= BLK] = np.inf
        b = int(np.argmin(score))
        block_of[r] = b
        pos_of[r] = cnt[b]
        cnt[b] += 1
        lo_load[b] += lo_deg[r]
        hi_load[b] += hi_deg[r]
    assert lo_load.max() <= lo_cap and hi_load.max() <= hi_cap, (
        lo_load.max(), hi_load.max())
    return block_of, pos_of


def prepare(x, edge_index, W, b, gamma, beta):
    x = np.asarray(x, np.float32)
    W = np.asarray(W, np.float32)
    gamma = np.asarray(gamma, np.float32)
    beta = np.asarray(beta, np.float32)
    N, D = x.shape
    assert N == N_NODES and D == DIM

    row = np.asarray(edge_index[0]).astype(np.int64)
    col = np.asarray(edge_index[1]).astype(np.int64)
    deg = (np.bincount(row, minlength=N) + 1).astype(np.float64)
    dinv = (deg ** -0.5).astype(np.float32)
    v = (dinv[:, None] * x).astype(ml_dtypes.bfloat16)
    x_bf = x.astype(ml_dtypes.bfloat16)

    core_of = row // NPC
    lrow = row - core_of * NPC
    is_hi = col >= SPLIT

    # per-row lo/hi degrees, per core
    lo_deg = np.bincount(row[~is_hi], minlength=N).reshape(CORES, NPC)
    hi_deg = np.bincount(row[is_hi], minlength=N).reshape(CORES, NPC)

    lo_idx = np.zeros((CORES, 128, NBLK * T_LO * 8), np.int16)
    hi_idx = np.zeros((CORES, 128, NBLK * T_HI * 8), np.int16)
    rel_arr = np.full((CORES, 128, NBLK * T), 200.0, np.float32)
    vlt = np.zeros((CORES, 128, NPC), ml_dtypes.bfloat16)
    dinvrow = np.zeros((CORES, 128, NPC), ml_dtypes.bfloat16)
    x_loc = np.zeros((CORES, BLK, NBLK * D), ml_dtypes.bfloat16)
    perms = []

    for k in range(CORES):
        blk_of, pos_of = _balance_blocks(
            lo_deg[k].astype(np.float64), hi_deg[k].astype(np.float64)
        )
        # perm[b*BLK + p] = local row index at (block b, pos p)
        perm = np.empty(NPC, np.int64)
        perm[blk_of * BLK + pos_of] = np.arange(NPC)
        perms.append(perm)
        grows = k * NPC + perm  # global row ids in device order
        vlt[k] = v[grows].T
        dinvrow[k] = np.tile(
            dinv[grows].astype(ml_dtypes.bfloat16)[None, :], (128, 1)
        )
        x_loc[k] = (
            x_bf[grows].reshape(NBLK, BLK, D).transpose(1, 0, 2).reshape(BLK, NBLK * D)
        )

        m = core_of == k
        ec, eb, ep, eh = col[m], blk_of[lrow[m]], pos_of[lrow[m]], is_hi[m]
        # sort edges by (block, lo/hi)
        skey = eb * 2 + eh
        order = np.argsort(skey, kind="stable")
        ec, eb, ep, eh = ec[order], eb[order], ep[order], eh[order]
        seg_cnt = np.bincount(eb * 2 + eh, minlength=NBLK * 2)
        seg_start = np.zeros(NBLK * 2 + 1, np.int64)
        np.cumsum(seg_cnt, out=seg_start[1:])
        pos_in_seg = np.arange(len(ec)) - seg_start[eb * 2 + eh]
        tile_in_blk = np.where(eh, T_LO + pos_in_seg // 128, pos_in_seg // 128)
        p_of = pos_in_seg % 128
        rel_arr[k, p_of, eb * T + tile_in_blk] = ep
        lo_segs, hi_segs = [], []
        for bb in range(NBLK):
            lo_vals = ec[seg_start[2 * bb]:seg_start[2 * bb + 1]]
            hi_vals = ec[seg_start[2 * bb + 1]:seg_start[2 * bb + 2]] - SPLIT
            lo_segs.append(lo_vals.astype(np.int16))
            hi_segs.append(hi_vals.astype(np.int16))
        lo_idx[k] = _pack_idx(lo_segs, T_LO, NBLK)
        hi_idx[k] = _pack_idx(hi_segs, T_HI, NBLK)

    rel_arr = rel_arr.astype(ml_dtypes.bfloat16)
    iota_rt = np.repeat(
        np.arange(BLK, dtype=np.float32), T
    )[None, :].repeat(128, 0).astype(ml_dtypes.bfloat16)
    gb = np.stack([gamma, beta], axis=1).astype(np.float32)
    w_bf = W.astype(ml_dtypes.bfloat16)

    in_maps = []
    for k in range(CORES):
        in_maps.append(
            {
                "v_tab": v,
                "lo_idx": lo_idx[k],
                "hi_idx": hi_idx[k],
                "rel_arr": rel_arr[k],
                "iota_rt": iota_rt,
                "v_loc_t": vlt[k],
                "dinvrow": dinvrow[k],
                "x_loc": x_loc[k],
                "w_mat": w_bf,
                "gb": gb,
            }
        )
    return perms, in_maps


def get_nc(params=None):
    if "nc" not in _CACHE:
        _CACHE["nc"] = _build_nc()
    return _CACHE["nc"]


def run(perms, in_maps, trace=False, **kw):
    from concourse.bass_utils import run_bass_kernel_spmd

    nc = get_nc()
    res = run_bass_kernel_spmd(nc, in_maps, list(range(CORES)), trace=trace, **kw)
    ys = []
    for k in range(CORES):
        yk = (
            np.asarray(res.results[k]["y_out"])
            .astype(np.float32)
            .reshape(BLK, NBLK, DIM)
            .transpose(1, 0, 2)
            .reshape(NPC, DIM)
        )
        inv = np.empty(NPC, np.int64)
        inv[perms[k]] = np.arange(NPC)
        ys.append(yk[inv])
    return np.concatenate(ys, axis=0), res


def kernel(x, edge_index, W, b, gamma, beta):
    perms, in_maps = prepare(x, edge_index, W, b, gamma, beta)
    y, _ = run(perms, in_maps)
    return y
